# revision 1
# baseline (speedup 1.0000x reference)
"""Multi-head self-attention (B=4, N=2048, C=512, H=8) on 8 trn2 NeuronCores.

Sharding: core = 2*b + g  (b = batch, g = head-half).  Each core handles one
batch element and 4 heads (channel slice of 256), computes its partial output
projection y^T = W_p[:, slice] @ out[slice], and the host sums the two
partials per batch element and adds b_proj.

Device algorithm per core (all matmuls fp32r, 1 cycle/row):
  1. qkv: q^T/k^T blocks (d on partitions) and v (tokens on partitions,
     packed [v | 1] per head for the softmax-denominator trick).
  2. attention per head: S^T = k^T.T @ q^T (keys on partitions), exp on ACT
     (scale folded in), out^T[65] = [v|1].T @ p^T accumulated over key tiles;
     row 64 = softmax denominator.  Normalize with reciprocal + gpsimd
     partition-broadcast + DVE multiply.
  3. projection: y^T accumulated over the two 128-channel blocks.
"""

import numpy as np

import concourse.bacc as bacc
import concourse.bass as bass
import concourse.mybir as mybir
import concourse.tile as tile
from concourse.bass_utils import run_bass_kernel_spmd

B, N, C, H, HD = 4, 2048, 512, 8, 64
HPC, CS = 4, 256  # heads per core, channels per core
SCALE = HD ** -0.5
F32R = mybir.dt.float32r
F32 = mybir.dt.float32
NCORES = 8
MT = N // 128  # 16 key tiles

# Set False to use per-head (K=64) projection without partition-shifted
# writes, if the packed head-pair layout ever miscompiles.
PACK_HEAD_PAIRS = True

_NC = None


def _build(reps=1):
    nc = bacc.Bacc("TRN2", target_bir_lowering=False, debug=False,
                   num_devices=NCORES)
    xT_d = nc.dram_tensor("xT", [C, N], F32R, kind="ExternalInput")
    wqT_d = nc.dram_tensor("wqT", [C, CS], F32R, kind="ExternalInput")
    wkT_d = nc.dram_tensor("wkT", [C, CS], F32R, kind="ExternalInput")
    wvT_d = nc.dram_tensor("wvT", [C, CS], F32R, kind="ExternalInput")
    wpT_d = nc.dram_tensor("wpT", [CS, C], F32R, kind="ExternalInput")
    bq_d = nc.dram_tensor("bq", [128, 2], F32, kind="ExternalInput")
    bk_d = nc.dram_tensor("bk", [128, 2], F32, kind="ExternalInput")
    bv_d = nc.dram_tensor("bv", [1, CS], F32R, kind="ExternalInput")
    ones_col_d = nc.dram_tensor("ones_col", [128, MT], F32R,
                                kind="ExternalInput")
    ones_row_d = nc.dram_tensor("ones_row", [1, 128], F32R,
                                kind="ExternalInput")
    yT_d = nc.dram_tensor("yT", [C, N], F32, kind="ExternalOutput")

    with tile.TileContext(nc) as tc:
      def body():
          with (
              tc.tile_pool(name="const", bufs=1) as const,
              tc.tile_pool(name="big", bufs=1) as big,
              tc.tile_pool(name="xt", bufs=1) as xtp,
              tc.tile_pool(name="pt", bufs=4) as ptp,
              tc.tile_pool(name="recip", bufs=2) as rcp,
              tc.tile_pool(name="rbc", bufs=2) as rbcp,
              tc.tile_pool(name="ysb", bufs=3) as ysbp,
          ):
              # ---- inputs: first key tiles + value weights race in on separate
              # DMA queues so the first matmul can start ~3us in --------------
              xt = [[xtp.tile([128, 512], F32R, tag=f"x{ct}_{nk}", name=f"x{ct}_{nk}")
                     for nk in range(4)] for ct in range(4)]
              wq_t, wk_t, wv_t = [], [], []
              for ct in range(4):
                  for lst, nm in ((wq_t, "wq"), (wk_t, "wk"), (wv_t, "wv")):
                      lst.append(const.tile([128, CS], F32R, tag=f"{nm}{ct}",
                                            name=f"{nm}{ct}"))
              for ct in range(4):
                  nc.sync.dma_start(out=xt[ct][0][:],
                                    in_=xT_d[bass.ts(ct, 128), bass.ts(0, 512)])
                  nc.gpsimd.dma_start(out=wv_t[ct][:],
                                      in_=wvT_d[bass.ts(ct, 128), :])
              bv_sb = const.tile([1, CS], F32R, tag="bv", name="bv")
              nc.gpsimd.dma_start(out=bv_sb[:], in_=bv_d[:])
              ones_row = const.tile([1, 128], F32R, tag="ones_row", name="ones_row")
              nc.gpsimd.dma_start(out=ones_row[:], in_=ones_row_d[:])
              for ct in range(4):
                  nc.gpsimd.dma_start(out=wq_t[ct][:],
                                      in_=wqT_d[bass.ts(ct, 128), :])
                  nc.gpsimd.dma_start(out=wk_t[ct][:],
                                      in_=wkT_d[bass.ts(ct, 128), :])
              for nk in range(1, 4):
                  for ct in range(4):
                      nc.sync.dma_start(
                          out=xt[ct][nk][:],
                          in_=xT_d[bass.ts(ct, 128), bass.ts(nk, 512)])
              wp_t = []
              for j in range(2):
                  t = const.tile([128, C], F32R, tag=f"wp{j}", name=f"wp{j}")
                  nc.gpsimd.dma_start(out=t[:], in_=wpT_d[bass.ts(j, 128), :])
                  wp_t.append(t)
              bq_sb = const.tile([128, 2], F32, tag="bq", name="bq")
              nc.gpsimd.dma_start(out=bq_sb[:], in_=bq_d[:])
              bk_sb = const.tile([128, 2], F32, tag="bk", name="bk")
              nc.gpsimd.dma_start(out=bk_sb[:], in_=bk_d[:])

              # ---- persistent activations -----------------------------------
              qT_c = [[big.tile([128, 512], F32R, tag=f"qT{j}_{nk}", name=f"qT{j}_{nk}")
                       for nk in range(4)] for j in range(2)]
              kT_c = [[big.tile([128, 512], F32R, tag=f"kT{j}_{nk}", name=f"kT{j}_{nk}")
                       for nk in range(4)] for j in range(2)]
              v1m = [big.tile([128, HPC, HD + 1], F32R, tag=f"v1m_{m}", name=f"v1m_{m}")
                     for m in range(MT)]
              for m in range(MT):
                  nc.gpsimd.dma_start(
                      out=v1m[m][:, :, HD:HD + 1],
                      in_=ones_col_d[:, 0:HPC].rearrange("p (h o) -> p h o", o=1),
                  )
              if PACK_HEAD_PAIRS:
                  oT_pair = [big.tile([128, N], F32R, tag=f"oT{j}", name=f"oT{j}")
                             for j in range(2)]
              else:
                  oT_head = [big.tile([64, N], F32R, tag=f"oTh{h}", name=f"oTh{h}")
                             for h in range(HPC)]



              # ---- phase 1: qkv ---------------------------------------------
              with (
                  tc.tile_pool(name="qkps", bufs=4, space="PSUM") as qkps,
                  tc.tile_pool(name="vps", bufs=2, space="PSUM") as vps,
              ):
                  for nk in range(4):
                      for nt in range(4):
                          m = nk * 4 + nt
                          vp = vps.tile([128, CS], F32, tag="v", name="v")
                          for ct in range(4):
                              nc.tensor.matmul(
                                  vp[:],
                                  lhsT=xt[ct][nk][:, bass.ts(nt, 128)],
                                  rhs=wv_t[ct][:],
                                  start=(ct == 0), stop=False,
                              )
                          nc.tensor.matmul(vp[:], lhsT=ones_row[:],
                                           rhs=bv_sb[:], start=False, stop=True)
                          nc.vector.tensor_copy(
                              v1m[m][:, :, 0:HD], vp[:])
                      for w_t, b_sb, dstC in ((wq_t, bq_sb, qT_c),
                                              (wk_t, bk_sb, kT_c)):
                          for j in range(2):
                              ps = qkps.tile([128, 512], F32, tag="qk", name="qk")
                              for ct in range(4):
                                  nc.tensor.matmul(
                                      ps[:],
                                      lhsT=w_t[ct][:, bass.ts(j, 128)],
                                      rhs=xt[ct][nk][:],
                                      start=(ct == 0), stop=(ct == 3),
                                  )
                              nc.vector.tensor_scalar_add(
                                  dstC[j][nk][:], ps[:],
                                  b_sb[:, j:j + 1],
                              )

              # ---- phase 2: attention ---------------------------------------
              with (
                  tc.tile_pool(name="stps", bufs=2, space="PSUM") as stps,
                  tc.tile_pool(name="otps", bufs=2, space="PSUM") as otps,
              ):
                  for h in range(HPC):
                      j, hh = h // 2, h % 2
                      psl = slice(hh * 64, hh * 64 + 64)
                      for p2 in range(2):
                          oT = otps.tile([HD + 1, 1024], F32, tag="oT", name="oT")
                          for m in range(MT):
                              sT = stps.tile([128, 1024], F32, tag="sT", name="sT")
                              for hf in range(2):
                                  nc.tensor.matmul(
                                      sT[:, bass.ts(hf, 512)],
                                      lhsT=kT_c[j][m // 4][psl, bass.ts(m % 4, 128)],
                                      rhs=qT_c[j][p2 * 2 + hf][psl, :],
                                      start=True, stop=True,
                                  )
                              pT = ptp.tile([128, 1024], F32R, tag="pT", name="pT")
                              nc.scalar.activation(
                                  out=pT[:], in_=sT[:],
                                  func=mybir.ActivationFunctionType.Exp,
                                  scale=SCALE,
                              )
                              for hf in range(2):
                                  nc.tensor.matmul(
                                      oT[:, bass.ts(hf, 512)],
                                      lhsT=v1m[m][:, h, :],
                                      rhs=pT[:, bass.ts(hf, 512)],
                                      start=(m == 0), stop=(m == MT - 1),
                                  )
                          rc = rcp.tile([1, 1024], F32, tag="rc", name="rc")
                          nc.vector.reciprocal(rc[:], oT[HD:HD + 1, :])
                          bc = rbcp.tile([64, 1024], F32, tag="bc", name="bc")
                          nc.gpsimd.partition_broadcast(bc[:], rc[:])
                          nsl = bass.ts(p2, 1024)
                          if PACK_HEAD_PAIRS:
                              nc.vector.tensor_mul(
                                  oT_pair[j][psl, nsl], oT[0:HD, :], bc[:])
                          else:
                              nc.vector.tensor_mul(
                                  oT_head[h][:, nsl], oT[0:HD, :], bc[:])

              # ---- phase 3: projection --------------------------------------
              with tc.tile_pool(name="yps", bufs=4, space="PSUM") as yps:
                  for jj in range(4):
                      ypt = [yps.tile([128, 512], F32, tag="yp", name="yp")
                             for _ in range(4)]
                      if PACK_HEAD_PAIRS:
                          for j in range(2):
                              for nk2 in range(4):
                                  nc.tensor.matmul(
                                      ypt[nk2][:],
                                      lhsT=wp_t[j][:, bass.ts(jj, 128)],
                                      rhs=oT_pair[j][:, bass.ts(nk2, 512)],
                                      start=(j == 0), stop=(j == 1),
                                  )
                      else:
                          for h in range(HPC):
                              wsl = slice(h % 2 * 64, h % 2 * 64 + 64)
                              for nk2 in range(4):
                                  nc.tensor.matmul(
                                      ypt[nk2][:],
                                      lhsT=wp_t[h // 2][wsl, bass.ts(jj, 128)],
                                      rhs=oT_head[h][:, bass.ts(nk2, 512)],
                                      start=(h == 0), stop=(h == HPC - 1),
                                  )
                      for nk2 in range(4):
                          ys = ysbp.tile([128, 512], F32, tag="ys", name="ys")
                          nc.vector.tensor_copy(ys[:], ypt[nk2][:])
                          nc.scalar.dma_start(
                              out=yT_d[bass.ts(jj, 128), bass.ts(nk2, 512)],
                              in_=ys[:],
                          )

      if reps > 1:
          with tc.For_i(0, reps, 1):
              body()
      else:
          body()

    nc.compile()
    return nc


def get_nc():
    global _NC
    if _NC is None:
        _NC = _build()
    return _NC


def build_timing_nc(reps):
    return _build(reps=reps)


def shard_inputs(x, w_qkv, b_qkv, w_proj, b_proj):
    x = np.asarray(x, dtype=np.float32)
    w_qkv = np.asarray(w_qkv, dtype=np.float32)
    b_qkv = np.asarray(b_qkv, dtype=np.float32)
    w_proj = np.asarray(w_proj, dtype=np.float32)
    ones_col = np.ones((128, MT), np.float32)
    ones_row = np.ones((1, 128), np.float32)
    in_maps = []
    for core in range(NCORES):
        b, g = core // 2, core % 2
        sl = slice(g * CS, (g + 1) * CS)
        in_maps.append({
            "xT": np.ascontiguousarray(x[b].T),
            "wqT": np.ascontiguousarray(w_qkv[sl, :].T),
            "wkT": np.ascontiguousarray(w_qkv[C:][sl, :].T),
            "wvT": np.ascontiguousarray(w_qkv[2 * C:][sl, :].T),
            "wpT": np.ascontiguousarray(w_proj[:, sl].T),
            "bq": np.ascontiguousarray(b_qkv[sl].reshape(2, 128).T),
            "bk": np.ascontiguousarray(b_qkv[C:][sl].reshape(2, 128).T),
            "bv": np.ascontiguousarray(b_qkv[2 * C:][sl].reshape(1, CS)),
            "ones_col": ones_col,
            "ones_row": ones_row,
        })
    return in_maps


def gather_output(results, b_proj):
    b_proj = np.asarray(b_proj, dtype=np.float32)
    out = np.empty((B, N, C), np.float32)
    for b in range(B):
        yT = results[2 * b]["yT"] + results[2 * b + 1]["yT"]
        out[b] = yT.T + b_proj[None, :]
    return out


def kernel(x, w_qkv, b_qkv, w_proj, b_proj):
    nc = get_nc()
    in_maps = shard_inputs(x, w_qkv, b_qkv, w_proj, b_proj)
    res = run_bass_kernel_spmd(nc, in_maps, core_ids=list(range(NCORES)))
    return gather_output(res.results, b_proj)



# revision 7
# speedup vs baseline: 1.6325x; 1.6325x over previous
"""Multi-head self-attention (B=4, N=2048, C=512, H=8) on 8 trn2 NeuronCores.

Sharding: core = 2*b + g  (b = batch, g = head-half).  Each core handles one
batch element and 4 heads (channel slice of 256), computes its partial output
projection y^T = W_p[:, slice] @ out[slice], and the host sums the two
partials per batch element and adds b_proj.

All matmul operands are fp16 (PSUM accumulation stays fp32); rel-err budget
is 2e-2 and fp16 keeps us ~1e-3.  fp16 streams 1 row/cycle on the PE vs
~2.5 cycles/row measured for fp32r.

Device algorithm per core:
  1. q^T/k^T: [128, 2048] tiles (channel pairs j=0,1 on partitions), single
     4-step accumulation over the 512 input channels; bias folded in the
     PSUM->SBUF copy.  v: [128 tok, 256 ch] tiles packed per head as
     [v | 1] for the softmax-denominator trick.
  2. attention per head h, query half p2: S^T = k^T.T @ q^T (keys on
     partitions, one 1024-wide matmul per key tile), exp on ACT (scale
     folded), out^T[65] = [v|1].T @ p^T accumulated over key tiles; row 64 =
     denominator.  Normalize: fast reciprocal + gpsimd partition-broadcast +
     DVE multiply.
  3. projection: y^T = W_p^T @ oT accumulated over the two 128-channel
     blocks, 2048-wide matmuls.
"""

import numpy as np

import concourse.bacc as bacc
import concourse.bass as bass
import concourse.mybir as mybir
import concourse.tile as tile
from concourse.bass_utils import run_bass_kernel_spmd

B, N, C, H, HD = 4, 2048, 512, 8, 64
HPC, CS = 4, 256  # heads per core, channels per core
SCALE = HD ** -0.5
F16 = mybir.dt.float16
F32 = mybir.dt.float32
NCORES = 8
MT = N // 128  # 16 key tiles

_NC = None


def _build():
    nc = bacc.Bacc("TRN2", target_bir_lowering=False, debug=False,
                   num_devices=NCORES)
    xT_d = nc.dram_tensor("xT", [C, N], F16, kind="ExternalInput")
    wqT_d = nc.dram_tensor("wqT", [C, CS], F16, kind="ExternalInput")
    wkT_d = nc.dram_tensor("wkT", [C, CS], F16, kind="ExternalInput")
    wvT_d = nc.dram_tensor("wvT", [C, CS], F16, kind="ExternalInput")
    wpT_d = nc.dram_tensor("wpT", [CS, C], F16, kind="ExternalInput")
    bq_d = nc.dram_tensor("bq", [128, 2], F32, kind="ExternalInput")
    bk_d = nc.dram_tensor("bk", [128, 2], F32, kind="ExternalInput")
    bv_d = nc.dram_tensor("bv", [1, CS], F16, kind="ExternalInput")
    ones_col_d = nc.dram_tensor("ones_col", [128, HPC], F16,
                                kind="ExternalInput")
    ones_row_d = nc.dram_tensor("ones_row", [1, 128], F16,
                                kind="ExternalInput")
    yT_d = nc.dram_tensor("yT", [C, N], F32, kind="ExternalOutput")

    with tile.TileContext(nc) as tc:
        with (
            tc.tile_pool(name="const", bufs=1) as const,
            tc.tile_pool(name="big", bufs=1) as big,
            tc.tile_pool(name="xt", bufs=1) as xtp,
            tc.tile_pool(name="pt", bufs=4) as ptp,
            tc.tile_pool(name="recip", bufs=2) as rcp,
            tc.tile_pool(name="rbc", bufs=2) as rbcp,
            tc.tile_pool(name="ysb", bufs=2) as ysbp,
        ):
            # ---- input DMAs: x split across two queues, weights on gpsimd
            xt = [xtp.tile([128, N], F16, tag=f"x{ct}", name=f"x{ct}")
                  for ct in range(4)]
            for ct in range(4):
                eng = nc.sync if ct % 2 == 0 else nc.scalar
                eng.dma_start(out=xt[ct][:], in_=xT_d[bass.ts(ct, 128), :])
            wq_t, wk_t, wv_t = [], [], []
            for lst, nm, src in ((wq_t, "wq", wqT_d), (wk_t, "wk", wkT_d),
                                 (wv_t, "wv", wvT_d)):
                for ct in range(4):
                    t = const.tile([128, CS], F16, tag=f"{nm}{ct}",
                                   name=f"{nm}{ct}")
                    nc.gpsimd.dma_start(out=t[:], in_=src[bass.ts(ct, 128), :])
                    lst.append(t)
            bq_sb = const.tile([128, 2], F32, tag="bq", name="bq")
            nc.gpsimd.dma_start(out=bq_sb[:], in_=bq_d[:])
            bk_sb = const.tile([128, 2], F32, tag="bk", name="bk")
            nc.gpsimd.dma_start(out=bk_sb[:], in_=bk_d[:])
            bv_sb = const.tile([1, CS], F16, tag="bv", name="bv")
            nc.gpsimd.dma_start(out=bv_sb[:], in_=bv_d[:])
            ones_row = const.tile([1, 128], F16, tag="ones_row",
                                  name="ones_row")
            nc.gpsimd.dma_start(out=ones_row[:], in_=ones_row_d[:])
            wp_t = []
            for j in range(2):
                t = const.tile([128, C], F16, tag=f"wp{j}", name=f"wp{j}")
                nc.gpsimd.dma_start(out=t[:], in_=wpT_d[bass.ts(j, 128), :])
                wp_t.append(t)

            # ---- persistent activations -------------------------------------
            qT_c = [big.tile([128, N], F16, tag=f"qT{j}", name=f"qT{j}")
                    for j in range(2)]
            kT_c = [big.tile([128, N], F16, tag=f"kT{j}", name=f"kT{j}")
                    for j in range(2)]
            v1m = [big.tile([128, HPC, HD + 1], F16, tag=f"v1m_{m}",
                            name=f"v1m_{m}") for m in range(MT)]
            for m in range(MT):
                nc.gpsimd.dma_start(
                    out=v1m[m][:, :, HD:HD + 1],
                    in_=ones_col_d[:, :].rearrange("p (h o) -> p h o", o=1),
                )
            oT_pair = [big.tile([128, N], F16, tag=f"oT{j}", name=f"oT{j}")
                       for j in range(2)]

            # ---- phase 1a: q^T, k^T -----------------------------------------
            with tc.tile_pool(name="kqps", bufs=2, space="PSUM") as kqps:
                for j in range(2):
                    for w_t, b_sb, dstC in ((wq_t, bq_sb, qT_c),
                                            (wk_t, bk_sb, kT_c)):
                        ps = kqps.tile([128, N], F32, tag="kq", name="kq")
                        for ct in range(4):
                            for nk in range(4):
                                nc.tensor.matmul(
                                    ps[:, bass.ts(nk, 512)],
                                    lhsT=w_t[ct][:, bass.ts(j, 128)],
                                    rhs=xt[ct][:, bass.ts(nk, 512)],
                                    start=(ct == 0), stop=(ct == 3),
                                )
                        nc.vector.tensor_scalar_add(
                            dstC[j][:], ps[:], b_sb[:, j:j + 1])

            # ---- phase 1b: v ------------------------------------------------
            with tc.tile_pool(name="vps", bufs=2, space="PSUM") as vps:
                for m in range(MT):
                    nk, nt = m // 4, m % 4
                    vp = vps.tile([128, CS], F32, tag="v", name="v")
                    for ct in range(4):
                        nc.tensor.matmul(
                            vp[:],
                            lhsT=xt[ct][:, bass.ts(m, 128)],
                            rhs=wv_t[ct][:],
                            start=(ct == 0), stop=False,
                        )
                    nc.tensor.matmul(vp[:], lhsT=ones_row[:], rhs=bv_sb[:],
                                     start=False, stop=True)
                    nc.vector.tensor_copy(v1m[m][:, :, 0:HD], vp[:])

            # ---- phase 2: attention -----------------------------------------
            with (
                tc.tile_pool(name="stps", bufs=2, space="PSUM") as stps,
                tc.tile_pool(name="otps", bufs=2, space="PSUM") as otps,
            ):
                for h in range(HPC):
                    j, hh = h // 2, h % 2
                    psl = slice(hh * 64, hh * 64 + 64)
                    for p2 in range(2):
                        nsl = bass.ts(p2, 1024)
                        oT = otps.tile([HD + 1, 1024], F32, tag="oT",
                                       name="oT")
                        for m in range(MT):
                            sT = stps.tile([128, 1024], F32, tag="sT",
                                           name="sT")
                            for hf in range(2):
                                nc.tensor.matmul(
                                    sT[:, bass.ts(hf, 512)],
                                    lhsT=kT_c[j][psl, bass.ts(m, 128)],
                                    rhs=qT_c[j][psl,
                                                p2 * 1024 + hf * 512:
                                                p2 * 1024 + hf * 512 + 512],
                                    start=True, stop=True,
                                )
                            pT = ptp.tile([128, 1024], F16, tag="pT",
                                          name="pT")
                            nc.scalar.activation(
                                out=pT[:], in_=sT[:],
                                func=mybir.ActivationFunctionType.Exp,
                                scale=SCALE,
                            )
                            for hf in range(2):
                                nc.tensor.matmul(
                                    oT[:, bass.ts(hf, 512)],
                                    lhsT=v1m[m][:, h, :],
                                    rhs=pT[:, bass.ts(hf, 512)],
                                    start=(m == 0), stop=(m == MT - 1),
                                )
                        den = rcp.tile([1, 1024], F32, tag="den", name="den")
                        nc.vector.tensor_copy(den[:], oT[HD:HD + 1, :])
                        rc = rcp.tile([1, 1024], F32, tag="rc", name="rc")
                        nc.vector.reciprocal_approx_fast(rc[:], den[:])
                        bc = rbcp.tile([64, 1024], F32, tag="bc", name="bc")
                        nc.gpsimd.partition_broadcast(bc[:], rc[:])
                        nc.vector.tensor_mul(
                            oT_pair[j][psl, nsl], oT[0:HD, :], bc[:])

            # ---- phase 3: projection ----------------------------------------
            with tc.tile_pool(name="yps", bufs=2, space="PSUM") as yps:
                for jj in range(4):
                    ypt = yps.tile([128, N], F32, tag="yp", name="yp")
                    for j in range(2):
                        for nk2 in range(4):
                            nc.tensor.matmul(
                                ypt[:, bass.ts(nk2, 512)],
                                lhsT=wp_t[j][:, bass.ts(jj, 128)],
                                rhs=oT_pair[j][:, bass.ts(nk2, 512)],
                                start=(j == 0), stop=(j == 1),
                            )
                    ys = ysbp.tile([128, N], F32, tag="ys", name="ys")
                    nc.vector.tensor_copy(ys[:], ypt[:])
                    nc.sync.dma_start(out=yT_d[bass.ts(jj, 128), :], in_=ys[:])

    nc.compile()
    return nc


def get_nc():
    global _NC
    if _NC is None:
        _NC = _build()
    return _NC


def shard_inputs(x, w_qkv, b_qkv, w_proj, b_proj):
    x = np.asarray(x, dtype=np.float32)
    w_qkv = np.asarray(w_qkv, dtype=np.float32)
    b_qkv = np.asarray(b_qkv, dtype=np.float32)
    w_proj = np.asarray(w_proj, dtype=np.float32)
    ones_col = np.ones((128, HPC), np.float16)
    ones_row = np.ones((1, 128), np.float16)
    in_maps = []
    for core in range(NCORES):
        b, g = core // 2, core % 2
        sl = slice(g * CS, (g + 1) * CS)
        in_maps.append({
            "xT": np.ascontiguousarray(x[b].T).astype(np.float16),
            "wqT": np.ascontiguousarray(w_qkv[sl, :].T).astype(np.float16),
            "wkT": np.ascontiguousarray(w_qkv[C:][sl, :].T).astype(np.float16),
            "wvT": np.ascontiguousarray(
                w_qkv[2 * C:][sl, :].T).astype(np.float16),
            "wpT": np.ascontiguousarray(w_proj[:, sl].T).astype(np.float16),
            "bq": np.ascontiguousarray(b_qkv[sl].reshape(2, 128).T),
            "bk": np.ascontiguousarray(b_qkv[C:][sl].reshape(2, 128).T),
            "bv": b_qkv[2 * C:][sl].reshape(1, CS).astype(np.float16),
            "ones_col": ones_col,
            "ones_row": ones_row,
        })
    return in_maps


def gather_output(results, b_proj):
    b_proj = np.asarray(b_proj, dtype=np.float32)
    out = np.empty((B, N, C), np.float32)
    for b in range(B):
        yT = results[2 * b]["yT"] + results[2 * b + 1]["yT"]
        out[b] = yT.T + b_proj[None, :]
    return out


def kernel(x, w_qkv, b_qkv, w_proj, b_proj):
    nc = get_nc()
    in_maps = shard_inputs(x, w_qkv, b_qkv, w_proj, b_proj)
    res = run_bass_kernel_spmd(nc, in_maps, core_ids=list(range(NCORES)))
    return gather_output(res.results, b_proj)


# revision 8
# speedup vs baseline: 1.6958x; 1.0388x over previous
"""Multi-head self-attention (B=4, N=2048, C=512, H=8) on 8 trn2 NeuronCores.

Sharding: core = 2*b + g  (b = batch, g = head-half).  Each core handles one
batch element and 4 heads (channel slice of 256), computes its partial output
projection y^T = W_p[:, slice] @ out[slice], and the host sums the two
partials per batch element (fp32) and adds b_proj.

All matmul operands are fp16 (PSUM accumulation fp32); the exp stream on the
ACT engine (128 x [128,1024] tiles, ~1.1us each) is the throughput floor, so
the emission order keeps ACT saturated from ~15us:

  kq_j0 (2 chunks each of q^T/k^T for heads 0-1)         [kqps pool, right]
  h0 p2=0 slots: scores+exp, v(m) tiles interleaved      [stps left, v in kqps]
  h0 p2=1 slots: scores+exp, kq_j1 chunks interleaved
  kqps releases -> otps opens (right)
  h1..h3 slots:  scores+exp + AV of the PREVIOUS unit (lag-1, deep pT ring)
  tail: AV(h3p2=0), AV(h3p2=1) back-to-back, chains, 2-pass projection
        (pass1 j=0 overlaps the last chains), fp16 output on 2 DMA queues.
"""

import numpy as np

import concourse.bacc as bacc
import concourse.bass as bass
import concourse.mybir as mybir
import concourse.tile as tile
from concourse.bass_utils import run_bass_kernel_spmd

B, N, C, H, HD = 4, 2048, 512, 8, 64
HPC, CS = 4, 256  # heads per core, channels per core
SCALE = HD ** -0.5
F16 = mybir.dt.float16
F32 = mybir.dt.float32
NCORES = 8
MT = N // 128  # 16 key tiles

_NC = None


def _build():
    nc = bacc.Bacc("TRN2", target_bir_lowering=False, debug=False,
                   num_devices=NCORES)
    xT_d = nc.dram_tensor("xT", [C, N], F16, kind="ExternalInput")
    wqT_d = nc.dram_tensor("wqT", [C, CS], F16, kind="ExternalInput")
    wkT_d = nc.dram_tensor("wkT", [C, CS], F16, kind="ExternalInput")
    wvT_d = nc.dram_tensor("wvT", [C, CS], F16, kind="ExternalInput")
    wpT_d = nc.dram_tensor("wpT", [CS, C], F16, kind="ExternalInput")
    bq_d = nc.dram_tensor("bq", [128, 2], F32, kind="ExternalInput")
    bk_d = nc.dram_tensor("bk", [128, 2], F32, kind="ExternalInput")
    bv_d = nc.dram_tensor("bv", [1, CS], F16, kind="ExternalInput")
    ones_col_d = nc.dram_tensor("ones_col", [128, HPC], F16,
                                kind="ExternalInput")
    ones_row_d = nc.dram_tensor("ones_row", [1, 128], F16,
                                kind="ExternalInput")
    yT_d = nc.dram_tensor("yT", [C, N], F16, kind="ExternalOutput")

    with tile.TileContext(nc) as tc:
        with (
            tc.tile_pool(name="const", bufs=1) as const,
            tc.tile_pool(name="big", bufs=1) as big,
            tc.tile_pool(name="xt", bufs=1) as xtp,
            tc.tile_pool(name="pt", bufs=24) as ptp,
            tc.tile_pool(name="recip", bufs=2) as rcp,
            tc.tile_pool(name="rbc", bufs=2) as rbcp,
            tc.tile_pool(name="ysb", bufs=4) as ysbp,
        ):
            # ---- input DMAs: x as 8 half-tiles on 2 queues (first halves
            # of all 4 ct tiles first, so kq chunk c0 can start early) ------
            xt = [xtp.tile([128, N], F16, tag=f"x{ct}", name=f"x{ct}")
                  for ct in range(4)]
            for half in range(2):
                for ct in range(4):
                    eng = nc.sync if ct % 2 == 0 else nc.scalar
                    eng.dma_start(
                        out=xt[ct][:, bass.ts(half, 1024)],
                        in_=xT_d[bass.ts(ct, 128), bass.ts(half, 1024)])
            wq_t, wk_t, wv_t = [], [], []
            for lst, nm, src in ((wq_t, "wq", wqT_d), (wk_t, "wk", wkT_d),
                                 (wv_t, "wv", wvT_d)):
                for ct in range(4):
                    t = const.tile([128, CS], F16, tag=f"{nm}{ct}",
                                   name=f"{nm}{ct}")
                    nc.gpsimd.dma_start(out=t[:], in_=src[bass.ts(ct, 128), :])
                    lst.append(t)
            bq_sb = const.tile([128, 2], F32, tag="bq", name="bq")
            nc.gpsimd.dma_start(out=bq_sb[:], in_=bq_d[:])
            bk_sb = const.tile([128, 2], F32, tag="bk", name="bk")
            nc.gpsimd.dma_start(out=bk_sb[:], in_=bk_d[:])
            bv_sb = const.tile([1, CS], F16, tag="bv", name="bv")
            nc.gpsimd.dma_start(out=bv_sb[:], in_=bv_d[:])
            ones_row = const.tile([1, 128], F16, tag="ones_row",
                                  name="ones_row")
            nc.gpsimd.dma_start(out=ones_row[:], in_=ones_row_d[:])
            wp_t = []
            for j in range(2):
                t = const.tile([128, C], F16, tag=f"wp{j}", name=f"wp{j}")
                nc.gpsimd.dma_start(out=t[:], in_=wpT_d[bass.ts(j, 128), :])
                wp_t.append(t)

            # ---- persistent activations ---------------------------------
            qT_c = [big.tile([128, N], F16, tag=f"qT{j}", name=f"qT{j}")
                    for j in range(2)]
            kT_c = [big.tile([128, N], F16, tag=f"kT{j}", name=f"kT{j}")
                    for j in range(2)]
            v1m = [big.tile([128, HPC, HD + 1], F16, tag=f"v1m_{m}",
                            name=f"v1m_{m}") for m in range(MT)]
            for m in range(MT):
                nc.gpsimd.dma_start(
                    out=v1m[m][:, :, HD:HD + 1],
                    in_=ones_col_d[:, :].rearrange("p (h o) -> p h o", o=1),
                )
            oT_pair = [big.tile([128, N], F16, tag=f"oT{j}", name=f"oT{j}")
                       for j in range(2)]
            y0 = [big.tile([128, N], F16, tag=f"y0_{jj}", name=f"y0_{jj}")
                  for jj in range(4)]

            # kq chunk: accumulate one [128,1024] column chunk of q^T or k^T
            # for head-pair j from the 4 input-channel tiles, add bias, cast.
            def kq_chunk(pool, w_t, b_sb, dst, j, c):
                ps = pool.tile([128, 1024], F32, tag="kq", name="kq")
                for ct in range(4):
                    for hf in range(2):
                        nc.tensor.matmul(
                            ps[:, bass.ts(hf, 512)],
                            lhsT=w_t[ct][:, bass.ts(j, 128)],
                            rhs=xt[ct][:, c * 1024 + hf * 512:
                                       c * 1024 + hf * 512 + 512],
                            start=(ct == 0), stop=(ct == 3),
                        )
                nc.vector.tensor_scalar_add(
                    dst[j][:, bass.ts(c, 1024)], ps[:], b_sb[:, j:j + 1])

            # v tile m: [128 tok, 256 ch] + bias, packed into v1m[m].
            def v_tile(pool, m):
                vp = pool.tile([128, 1024], F32, tag="kq", name="kq")
                for ct in range(4):
                    nc.tensor.matmul(
                        vp[:, 0:CS],
                        lhsT=xt[ct][:, bass.ts(m, 128)],
                        rhs=wv_t[ct][:],
                        start=(ct == 0), stop=False,
                    )
                nc.tensor.matmul(vp[:, 0:CS], lhsT=ones_row[:], rhs=bv_sb[:],
                                 start=False, stop=True)
                nc.vector.tensor_copy(v1m[m][:, :, 0:HD], vp[:, 0:CS])

            def scores_exp(h, p2, m):
                j, hh = h // 2, h % 2
                psl = slice(hh * 64, hh * 64 + 64)
                sT = stps.tile([128, 1024], F32, tag="sT", name="sT")
                for hf in range(2):
                    nc.tensor.matmul(
                        sT[:, bass.ts(hf, 512)],
                        lhsT=kT_c[j][psl, bass.ts(m, 128)],
                        rhs=qT_c[j][psl, p2 * 1024 + hf * 512:
                                    p2 * 1024 + hf * 512 + 512],
                        start=True, stop=True,
                    )
                pT = ptp.tile([128, 1024], F16, tag="pT", name="pT")
                nc.scalar.activation(
                    out=pT[:], in_=sT[:],
                    func=mybir.ActivationFunctionType.Exp,
                    scale=SCALE,
                )
                return pT

            def av(oT, h, m, pT, first, last):
                for hf in range(2):
                    nc.tensor.matmul(
                        oT[:, bass.ts(hf, 512)],
                        lhsT=v1m[m][:, h, :],
                        rhs=pT[:, bass.ts(hf, 512)],
                        start=first, stop=last,
                    )

            def chain(oT, h, p2):
                j, hh = h // 2, h % 2
                psl = slice(hh * 64, hh * 64 + 64)
                den = rcp.tile([1, 1024], F32, tag="den", name="den")
                nc.vector.tensor_copy(den[:], oT[HD:HD + 1, :])
                rc = rcp.tile([1, 1024], F32, tag="rc", name="rc")
                nc.vector.reciprocal_approx_fast(rc[:], den[:])
                bc = rbcp.tile([64, 1024], F32, tag="bc", name="bc")
                nc.gpsimd.partition_broadcast(bc[:], rc[:])
                nc.vector.tensor_mul(
                    oT_pair[j][psl, bass.ts(p2, 1024)], oT[0:HD, :], bc[:])

            # units in order: (h, p2) for h in 0..3, p2 in 0..1
            units = [(h, p2) for h in range(HPC) for p2 in range(2)]
            pT_ring = {}  # unit index -> list of pT tiles
            oT_of = {}

            kqps = tc.alloc_tile_pool(name="kqps", bufs=2, space="PSUM",
                                      side="right")
            # q/k for heads 0-1, column chunk 0 first (unblocks h0 p2=0)
            kq_chunk(kqps, wq_t, bq_sb, qT_c, 0, 0)
            kq_chunk(kqps, wk_t, bk_sb, kT_c, 0, 0)
            kq_chunk(kqps, wq_t, bq_sb, qT_c, 0, 1)
            kq_chunk(kqps, wk_t, bk_sb, kT_c, 0, 1)

            stps = tc.alloc_tile_pool(name="stps", bufs=2, space="PSUM",
                                      side="left")
            # ---- unit 0 (h0, p2=0): scores/exp + v tiles interleaved -----
            pT_ring[0] = []
            for m in range(MT):
                pT_ring[0].append(scores_exp(0, 0, m))
                v_tile(kqps, m)
            # ---- unit 1 (h0, p2=1): scores/exp + kq_j1 chunks ------------
            pT_ring[1] = []
            kq1 = [(wq_t, bq_sb, qT_c, 1, 0), (wk_t, bk_sb, kT_c, 1, 0),
                   (wq_t, bq_sb, qT_c, 1, 1), (wk_t, bk_sb, kT_c, 1, 1)]
            for m in range(MT):
                pT_ring[1].append(scores_exp(0, 1, m))
                if m % 4 == 1:
                    w_t, b_sb, dst, j, c = kq1[m // 4]
                    kq_chunk(kqps, w_t, b_sb, dst, j, c)
            kqps.release()

            otps = tc.alloc_tile_pool(name="otps", bufs=2, space="PSUM",
                                      side="right")
            # ---- units 2..7: scores/exp + AV of unit u-2 (lag) -----------
            # unit u's scores stream while unit u-2's AVs drain; chains fire
            # as each drained unit completes.
            for u in range(2, 8):
                h, p2 = units[u]
                ph, pp2 = units[u - 2]
                oT_prev = otps.tile([HD + 1, 1024], F32, tag="oT",
                                    name="oT")
                oT_of[u - 2] = oT_prev
                pT_ring[u] = []
                for m in range(MT):
                    pT_ring[u].append(scores_exp(h, p2, m))
                    av(oT_prev, ph, m, pT_ring[u - 2][m],
                       first=(m == 0), last=(m == MT - 1))
                chain(oT_prev, ph, pp2)
            stps.release()

            # ---- tail: AVs for units 6,7 back-to-back, chains, projection
            yps = tc.alloc_tile_pool(name="yps", bufs=2, space="PSUM",
                                     side="left")
            for u in (6, 7):
                h, p2 = units[u]
                oT = otps.tile([HD + 1, 1024], F32, tag="oT", name="oT")
                for m in range(MT):
                    av(oT, h, m, pT_ring[u][m],
                       first=(m == 0), last=(m == MT - 1))
                chain(oT, h, p2)
                if u == 6:
                    # projection pass 1 (j=0): overlaps unit 7's AVs/chain
                    for jj in range(4):
                        for c2 in range(4):
                            yp = yps.tile([128, 512], F32, tag="yp",
                                          name="yp")
                            nc.tensor.matmul(
                                yp[:],
                                lhsT=wp_t[0][:, bass.ts(jj, 128)],
                                rhs=oT_pair[0][:, bass.ts(c2, 512)],
                                start=True, stop=True,
                            )
                            nc.vector.tensor_copy(
                                y0[jj][:, bass.ts(c2, 512)], yp[:])
            otps.release()

            # projection pass 2 (j=1) + output
            for jj in range(4):
                for c2 in range(4):
                    yp = yps.tile([128, 512], F32, tag="yp", name="yp")
                    nc.tensor.matmul(
                        yp[:],
                        lhsT=wp_t[1][:, bass.ts(jj, 128)],
                        rhs=oT_pair[1][:, bass.ts(c2, 512)],
                        start=True, stop=True,
                    )
                    ys = ysbp.tile([128, 512], F16, tag="ys", name="ys")
                    nc.vector.tensor_add(
                        ys[:], y0[jj][:, bass.ts(c2, 512)], yp[:])
                    eng = nc.sync if c2 % 2 == 0 else nc.scalar
                    eng.dma_start(
                        out=yT_d[bass.ts(jj, 128), bass.ts(c2, 512)],
                        in_=ys[:])
            yps.release()

    nc.compile()
    return nc


def get_nc():
    global _NC
    if _NC is None:
        _NC = _build()
    return _NC


def shard_inputs(x, w_qkv, b_qkv, w_proj, b_proj):
    x = np.asarray(x, dtype=np.float32)
    w_qkv = np.asarray(w_qkv, dtype=np.float32)
    b_qkv = np.asarray(b_qkv, dtype=np.float32)
    w_proj = np.asarray(w_proj, dtype=np.float32)
    ones_col = np.ones((128, HPC), np.float16)
    ones_row = np.ones((1, 128), np.float16)
    in_maps = []
    for core in range(NCORES):
        b, g = core // 2, core % 2
        sl = slice(g * CS, (g + 1) * CS)
        in_maps.append({
            "xT": np.ascontiguousarray(x[b].T).astype(np.float16),
            "wqT": np.ascontiguousarray(w_qkv[sl, :].T).astype(np.float16),
            "wkT": np.ascontiguousarray(w_qkv[C:][sl, :].T).astype(np.float16),
            "wvT": np.ascontiguousarray(
                w_qkv[2 * C:][sl, :].T).astype(np.float16),
            "wpT": np.ascontiguousarray(w_proj[:, sl].T).astype(np.float16),
            "bq": np.ascontiguousarray(b_qkv[sl].reshape(2, 128).T),
            "bk": np.ascontiguousarray(b_qkv[C:][sl].reshape(2, 128).T),
            "bv": b_qkv[2 * C:][sl].reshape(1, CS).astype(np.float16),
            "ones_col": ones_col,
            "ones_row": ones_row,
        })
    return in_maps


def gather_output(results, b_proj):
    b_proj = np.asarray(b_proj, dtype=np.float32)
    out = np.empty((B, N, C), np.float32)
    for b in range(B):
        yT = (results[2 * b]["yT"].astype(np.float32)
              + results[2 * b + 1]["yT"].astype(np.float32))
        out[b] = yT.T + b_proj[None, :]
    return out


def kernel(x, w_qkv, b_qkv, w_proj, b_proj):
    nc = get_nc()
    in_maps = shard_inputs(x, w_qkv, b_qkv, w_proj, b_proj)
    res = run_bass_kernel_spmd(nc, in_maps, core_ids=list(range(NCORES)))
    return gather_output(res.results, b_proj)


# revision 9
# speedup vs baseline: 1.7301x; 1.0203x over previous
"""Multi-head self-attention (B=4, N=2048, C=512, H=8) on 8 trn2 NeuronCores.

Sharding: core = 2*b + g  (b = batch, g = head-half).  Each core handles one
batch element and 4 heads (channel slice of 256), computes its partial output
projection y^T = W_p[:, slice] @ out[slice], and the host sums the two
partials per batch element (fp32) and adds b_proj.

All matmul operands are fp16 (PSUM accumulation fp32).  The exp stream on
the ACT engine (128 x [128,1024] tiles, ~1.1us each) is the throughput
floor; emission keeps ACT saturated from ~15us and the PE continuously fed:

  q/k head-pair 0, column chunk 0 only               [kqps pool, PSUM right]
  unit (h0,p2=0): scores+exp, fillers: kq_j0 chunk 1 + v tiles
  unit (h0,p2=1): scores+exp, fillers: kq_j1 chunks  [all in kqps tiles]
  kqps releases -> otps opens (right)
  units 2..7: scores+exp + AV of unit u-2 (lag-2, deep pT ring)
  tail: AV(u6)+chain, AV(u7)+chain, then 2-pass projection: pass1 j=0
  (ACT Copy to SBUF - same ACT table as Exp), pass2 j=1 (DVE add), fp16
  output on 2 DMA queues.
"""

import numpy as np

import concourse.bacc as bacc
import concourse.bass as bass
import concourse.mybir as mybir
import concourse.tile as tile
from concourse.bass_utils import run_bass_kernel_spmd

B, N, C, H, HD = 4, 2048, 512, 8, 64
HPC, CS = 4, 256  # heads per core, channels per core
SCALE = HD ** -0.5
F16 = mybir.dt.float16
F32 = mybir.dt.float32
NCORES = 8
MT = N // 128  # 16 key tiles

_NC = None


def _build():
    nc = bacc.Bacc("TRN2", target_bir_lowering=False, debug=False,
                   num_devices=NCORES)
    xT_d = nc.dram_tensor("xT", [C, N], F16, kind="ExternalInput")
    wqT_d = nc.dram_tensor("wqT", [C, CS], F16, kind="ExternalInput")
    wkT_d = nc.dram_tensor("wkT", [C, CS], F16, kind="ExternalInput")
    wvT_d = nc.dram_tensor("wvT", [C, CS], F16, kind="ExternalInput")
    wpT_d = nc.dram_tensor("wpT", [CS, C], F16, kind="ExternalInput")
    bq_d = nc.dram_tensor("bq", [128, 2], F32, kind="ExternalInput")
    bk_d = nc.dram_tensor("bk", [128, 2], F32, kind="ExternalInput")
    bv_d = nc.dram_tensor("bv", [1, CS], F16, kind="ExternalInput")
    ones_col_d = nc.dram_tensor("ones_col", [128, HPC], F16,
                                kind="ExternalInput")
    yT_d = nc.dram_tensor("yT", [C, N], F16, kind="ExternalOutput")

    with tile.TileContext(nc) as tc:
        with (
            tc.tile_pool(name="const", bufs=1) as const,
            tc.tile_pool(name="big", bufs=1) as big,
            tc.tile_pool(name="xt", bufs=1) as xtp,
            tc.tile_pool(name="pt", bufs=24) as ptp,
            tc.tile_pool(name="recip", bufs=2) as rcp,
            tc.tile_pool(name="rbc", bufs=2) as rbcp,
            tc.tile_pool(name="ysb", bufs=4) as ysbp,
        ):
            # ---- input DMAs: x as 8 half-tiles on 2 queues (first halves
            # of all 4 ct tiles first, so the kq c0 chunks start early) ----
            xt = [xtp.tile([128, N], F16, tag=f"x{ct}", name=f"x{ct}")
                  for ct in range(4)]
            for half in range(2):
                for ct in range(4):
                    eng = nc.sync if ct % 2 == 0 else nc.scalar
                    eng.dma_start(
                        out=xt[ct][:, bass.ts(half, 1024)],
                        in_=xT_d[bass.ts(ct, 128), bass.ts(half, 1024)])
            wq_t, wk_t, wv_t = [], [], []
            for lst, nm, src in ((wq_t, "wq", wqT_d), (wk_t, "wk", wkT_d),
                                 (wv_t, "wv", wvT_d)):
                for ct in range(4):
                    t = const.tile([128, CS], F16, tag=f"{nm}{ct}",
                                   name=f"{nm}{ct}")
                    nc.gpsimd.dma_start(out=t[:], in_=src[bass.ts(ct, 128), :])
                    lst.append(t)
            bq_sb = const.tile([128, 2], F32, tag="bq", name="bq")
            nc.gpsimd.dma_start(out=bq_sb[:], in_=bq_d[:])
            bk_sb = const.tile([128, 2], F32, tag="bk", name="bk")
            nc.gpsimd.dma_start(out=bk_sb[:], in_=bk_d[:])
            bv_sb = const.tile([1, CS], F16, tag="bv", name="bv")
            nc.gpsimd.dma_start(out=bv_sb[:], in_=bv_d[:])
            wp_t = []
            for j in range(2):
                t = const.tile([128, C], F16, tag=f"wp{j}", name=f"wp{j}")
                nc.gpsimd.dma_start(out=t[:], in_=wpT_d[bass.ts(j, 128), :])
                wp_t.append(t)
            # v bias broadcast to all partitions once (gpsimd, off the
            # critical path) so v tiles need no bias matmul.
            bvb = const.tile([128, CS], F16, tag="bvb", name="bvb")
            nc.gpsimd.partition_broadcast(bvb[:], bv_sb[:])

            # ---- persistent activations ---------------------------------
            qT_c = [big.tile([128, N], F16, tag=f"qT{j}", name=f"qT{j}")
                    for j in range(2)]
            kT_c = [big.tile([128, N], F16, tag=f"kT{j}", name=f"kT{j}")
                    for j in range(2)]
            v1m = [big.tile([128, HPC, HD + 1], F16, tag=f"v1m_{m}",
                            name=f"v1m_{m}") for m in range(MT)]
            for m in range(MT):
                nc.gpsimd.dma_start(
                    out=v1m[m][:, :, HD:HD + 1],
                    in_=ones_col_d[:, :].rearrange("p (h o) -> p h o", o=1),
                )
            oT_pair = [big.tile([128, N], F16, tag=f"oT{j}", name=f"oT{j}")
                       for j in range(2)]
            y0 = [big.tile([128, N], F16, tag=f"y0_{jj}", name=f"y0_{jj}")
                  for jj in range(4)]

            # kq chunk: one [128,1024] column chunk of q^T or k^T for head
            # pair j, accumulated over the 4 input-channel tiles + bias.
            def kq_chunk(pool, w_t, b_sb, dst, j, c):
                ps = pool.tile([128, 1024], F32, tag="kq", name="kq")
                for ct in range(4):
                    for hf in range(2):
                        nc.tensor.matmul(
                            ps[:, bass.ts(hf, 512)],
                            lhsT=w_t[ct][:, bass.ts(j, 128)],
                            rhs=xt[ct][:, c * 1024 + hf * 512:
                                       c * 1024 + hf * 512 + 512],
                            start=(ct == 0), stop=(ct == 3),
                        )
                nc.vector.tensor_scalar_add(
                    dst[j][:, bass.ts(c, 1024)], ps[:], b_sb[:, j:j + 1])

            # same as kq_chunk but split into 5 filler granules (4x 2-MM
            # accumulation steps + the bias/cast copy)
            def kq_granules(pool, w_t, b_sb, dst, j, c):
                st = {}
                def mk(ct):
                    def f():
                        if ct == 0:
                            st['ps'] = pool.tile([128, 1024], F32, tag="kq",
                                                 name="kq")
                        for hf in range(2):
                            nc.tensor.matmul(
                                st['ps'][:, bass.ts(hf, 512)],
                                lhsT=w_t[ct][:, bass.ts(j, 128)],
                                rhs=xt[ct][:, c * 1024 + hf * 512:
                                           c * 1024 + hf * 512 + 512],
                                start=(ct == 0), stop=(ct == 3),
                            )
                    return f
                def add():
                    nc.vector.tensor_scalar_add(
                        dst[j][:, bass.ts(c, 1024)], st['ps'][:],
                        b_sb[:, j:j + 1])
                return [mk(0), mk(1), mk(2), mk(3), add]

            def v_granule(pool, m):
                def f():
                    vp = pool.tile([128, 1024], F32, tag="kq", name="kq")
                    for ct in range(4):
                        nc.tensor.matmul(
                            vp[:, 0:CS],
                            lhsT=xt[ct][:, bass.ts(m, 128)],
                            rhs=wv_t[ct][:],
                            start=(ct == 0), stop=(ct == 3),
                        )
                    nc.vector.tensor_add(v1m[m][:, :, 0:HD], vp[:, 0:CS],
                                         bvb[:])
                return f

            def scores_exp(h, p2, m):
                j, hh = h // 2, h % 2
                psl = slice(hh * 64, hh * 64 + 64)
                sT = stps.tile([128, 1024], F32, tag="sT", name="sT")
                for hf in range(2):
                    nc.tensor.matmul(
                        sT[:, bass.ts(hf, 512)],
                        lhsT=kT_c[j][psl, bass.ts(m, 128)],
                        rhs=qT_c[j][psl, p2 * 1024 + hf * 512:
                                    p2 * 1024 + hf * 512 + 512],
                        start=True, stop=True,
                    )
                pT = ptp.tile([128, 1024], F16, tag="pT", name="pT")
                nc.scalar.activation(
                    out=pT[:], in_=sT[:],
                    func=mybir.ActivationFunctionType.Exp,
                    scale=SCALE,
                )
                return pT

            def av(oT, h, m, pT, first, last):
                for hf in range(2):
                    nc.tensor.matmul(
                        oT[:, bass.ts(hf, 512)],
                        lhsT=v1m[m][:, h, :],
                        rhs=pT[:, bass.ts(hf, 512)],
                        start=first, stop=last,
                    )

            def chain(oT, h, p2):
                j, hh = h // 2, h % 2
                psl = slice(hh * 64, hh * 64 + 64)
                den = rcp.tile([1, 1024], F32, tag="den", name="den")
                nc.vector.tensor_copy(den[:], oT[HD:HD + 1, :])
                rc = rcp.tile([1, 1024], F32, tag="rc", name="rc")
                nc.vector.reciprocal_approx_fast(rc[:], den[:])
                bc = rbcp.tile([64, 1024], F32, tag="bc", name="bc")
                nc.gpsimd.partition_broadcast(bc[:], rc[:])
                nc.vector.tensor_mul(
                    oT_pair[j][psl, bass.ts(p2, 1024)], oT[0:HD, :], bc[:])

            units = [(h, p2) for h in range(HPC) for p2 in range(2)]
            pT_ring = {}

            kqps = tc.alloc_tile_pool(name="kqps", bufs=2, space="PSUM",
                                      side="right")
            kq_chunk(kqps, wq_t, bq_sb, qT_c, 0, 0)
            kq_chunk(kqps, wk_t, bk_sb, kT_c, 0, 0)

            stps = tc.alloc_tile_pool(name="stps", bufs=2, space="PSUM",
                                      side="left")
            # ---- unit 0 (h0,p2=0): fillers = kq_j0 chunk 1 + v tiles ----
            fillers = (kq_granules(kqps, wq_t, bq_sb, qT_c, 0, 1)
                       + kq_granules(kqps, wk_t, bk_sb, kT_c, 0, 1)
                       + [v_granule(kqps, m) for m in range(MT)])
            pT_ring[0] = []
            for m in range(MT):
                pT_ring[0].append(scores_exp(0, 0, m))
                npop = 1 if m < 6 else 2
                for _ in range(npop):
                    if fillers:
                        fillers.pop(0)()
            # ---- unit 1 (h0,p2=1): fillers = kq_j1 (all 4 chunks) -------
            fillers += (kq_granules(kqps, wq_t, bq_sb, qT_c, 1, 0)
                        + kq_granules(kqps, wk_t, bk_sb, kT_c, 1, 0)
                        + kq_granules(kqps, wq_t, bq_sb, qT_c, 1, 1)
                        + kq_granules(kqps, wk_t, bk_sb, kT_c, 1, 1))
            pT_ring[1] = []
            for m in range(MT):
                pT_ring[1].append(scores_exp(0, 1, m))
                npop = 1 if m < 10 else 3
                for _ in range(npop):
                    if fillers:
                        fillers.pop(0)()
            while fillers:
                fillers.pop(0)()
            kqps.release()

            otps = tc.alloc_tile_pool(name="otps", bufs=2, space="PSUM",
                                      side="right")
            # ---- units 2..7: scores/exp + AV of unit u-2 (lag) ----------
            for u in range(2, 8):
                h, p2 = units[u]
                ph, pp2 = units[u - 2]
                oT_prev = otps.tile([HD + 1, 1024], F32, tag="oT", name="oT")
                pT_ring[u] = []
                for m in range(MT):
                    pT_ring[u].append(scores_exp(h, p2, m))
                    av(oT_prev, ph, m, pT_ring[u - 2][m],
                       first=(m == 0), last=(m == MT - 1))
                chain(oT_prev, ph, pp2)
            stps.release()

            # ---- tail: AVs for units 6,7, chains, 2-pass projection -----
            for u in (6, 7):
                h, p2 = units[u]
                oT = otps.tile([HD + 1, 1024], F32, tag="oT", name="oT")
                for m in range(MT):
                    av(oT, h, m, pT_ring[u][m],
                       first=(m == 0), last=(m == MT - 1))
                chain(oT, h, p2)
            otps.release()

            yps = tc.alloc_tile_pool(name="yps", bufs=4, space="PSUM",
                                     side="left")
            # pass 1 (j=0): MM + ACT Copy (same ACT table as Exp, ACT idle)
            for jj in range(4):
                for c2 in range(4):
                    yp = yps.tile([128, 512], F32, tag="yp", name="yp")
                    nc.tensor.matmul(
                        yp[:],
                        lhsT=wp_t[0][:, bass.ts(jj, 128)],
                        rhs=oT_pair[0][:, bass.ts(c2, 512)],
                        start=True, stop=True,
                    )
                    nc.scalar.activation(
                        out=y0[jj][:, bass.ts(c2, 512)], in_=yp[:],
                        func=mybir.ActivationFunctionType.Copy,
                    )
            # pass 2 (j=1): MM + DVE add + DMA out on 2 queues
            for jj in range(4):
                for c2 in range(4):
                    yp = yps.tile([128, 512], F32, tag="yp", name="yp")
                    nc.tensor.matmul(
                        yp[:],
                        lhsT=wp_t[1][:, bass.ts(jj, 128)],
                        rhs=oT_pair[1][:, bass.ts(c2, 512)],
                        start=True, stop=True,
                    )
                    ys = ysbp.tile([128, 512], F16, tag="ys", name="ys")
                    nc.vector.tensor_add(
                        ys[:], y0[jj][:, bass.ts(c2, 512)], yp[:])
                    eng = nc.sync if c2 % 2 == 0 else nc.scalar
                    eng.dma_start(
                        out=yT_d[bass.ts(jj, 128), bass.ts(c2, 512)],
                        in_=ys[:])
            yps.release()

    nc.compile()
    return nc


def get_nc():
    global _NC
    if _NC is None:
        _NC = _build()
    return _NC


def shard_inputs(x, w_qkv, b_qkv, w_proj, b_proj):
    x = np.asarray(x, dtype=np.float32)
    w_qkv = np.asarray(w_qkv, dtype=np.float32)
    b_qkv = np.asarray(b_qkv, dtype=np.float32)
    w_proj = np.asarray(w_proj, dtype=np.float32)
    ones_col = np.ones((128, HPC), np.float16)
    in_maps = []
    for core in range(NCORES):
        b, g = core // 2, core % 2
        sl = slice(g * CS, (g + 1) * CS)
        in_maps.append({
            "xT": np.ascontiguousarray(x[b].T).astype(np.float16),
            "wqT": np.ascontiguousarray(w_qkv[sl, :].T).astype(np.float16),
            "wkT": np.ascontiguousarray(w_qkv[C:][sl, :].T).astype(np.float16),
            "wvT": np.ascontiguousarray(
                w_qkv[2 * C:][sl, :].T).astype(np.float16),
            "wpT": np.ascontiguousarray(w_proj[:, sl].T).astype(np.float16),
            "bq": np.ascontiguousarray(b_qkv[sl].reshape(2, 128).T),
            "bk": np.ascontiguousarray(b_qkv[C:][sl].reshape(2, 128).T),
            "bv": b_qkv[2 * C:][sl].reshape(1, CS).astype(np.float16),
            "ones_col": ones_col,
        })
    return in_maps


def gather_output(results, b_proj):
    b_proj = np.asarray(b_proj, dtype=np.float32)
    out = np.empty((B, N, C), np.float32)
    for b in range(B):
        yT = (results[2 * b]["yT"].astype(np.float32)
              + results[2 * b + 1]["yT"].astype(np.float32))
        out[b] = yT.T + b_proj[None, :]
    return out


def kernel(x, w_qkv, b_qkv, w_proj, b_proj):
    nc = get_nc()
    in_maps = shard_inputs(x, w_qkv, b_qkv, w_proj, b_proj)
    res = run_bass_kernel_spmd(nc, in_maps, core_ids=list(range(NCORES)))
    return gather_output(res.results, b_proj)


# revision 21
# speedup vs baseline: 1.7401x; 1.0058x over previous
"""Multi-head self-attention (B=4, N=2048, C=512, H=8) on 8 trn2 NeuronCores.

Sharding: core = 2*b + g  (b = batch, g = head-half).  Each core handles one
batch element and 4 heads (channel slice of 256), computes its partial output
projection y^T = W_p[:, slice] @ out[slice], and the host sums the two
partials per batch element (fp32) and adds b_proj.

All matmul operands are fp16 (PSUM accumulation fp32).  The exp stream on
the ACT engine (128 x [128,1024] tiles, ~1.1us each) is the throughput
floor; emission keeps ACT saturated from ~15us and the PE continuously fed:

  q/k head-pair 0, column chunk 0 only               [kqps pool, PSUM right]
  unit (h0,p2=0): scores+exp, fillers: kq_j0 chunk 1 + v tiles
  unit (h0,p2=1): scores+exp, fillers: kq_j1 chunks  [all in kqps tiles]
  kqps releases -> otps opens (right)
  units 2..7: scores+exp + AV of unit u-2 (lag-2, deep pT ring)
  tail: AV(u6)+chain, AV(u7)+chain, then 2-pass projection: pass1 j=0
  (ACT Copy to SBUF - same ACT table as Exp), pass2 j=1 (DVE add), fp16
  output on 2 DMA queues.
"""

import numpy as np

import concourse.bacc as bacc
import concourse.bass as bass
import concourse.mybir as mybir
import concourse.tile as tile
from concourse.bass_utils import run_bass_kernel_spmd

B, N, C, H, HD = 4, 2048, 512, 8, 64
HPC, CS = 4, 256  # heads per core, channels per core
SCALE = HD ** -0.5
F16 = mybir.dt.float16
F32 = mybir.dt.float32
NCORES = 8
MT = N // 128  # 16 key tiles

_NC = None


def _build():
    nc = bacc.Bacc("TRN2", target_bir_lowering=False, debug=False,
                   num_devices=NCORES)
    xT_d = nc.dram_tensor("xT", [C, N], F16, kind="ExternalInput")
    wqT_d = nc.dram_tensor("wqT", [C, CS], F16, kind="ExternalInput")
    wkT_d = nc.dram_tensor("wkT", [C, CS], F16, kind="ExternalInput")
    wvT_d = nc.dram_tensor("wvT", [C, CS], F16, kind="ExternalInput")
    wpT_d = nc.dram_tensor("wpT", [CS, C], F16, kind="ExternalInput")
    bq_d = nc.dram_tensor("bq", [128, 2], F32, kind="ExternalInput")
    bk_d = nc.dram_tensor("bk", [128, 2], F32, kind="ExternalInput")
    bvb_d = nc.dram_tensor("bvb", [128, CS], F16, kind="ExternalInput")
    ones_col_d = nc.dram_tensor("ones_col", [128, HPC], F16,
                                kind="ExternalInput")
    yT_d = nc.dram_tensor("yT", [C, N], F16, kind="ExternalOutput")

    with tile.TileContext(nc) as tc:
        with (
            tc.tile_pool(name="const", bufs=1) as const,
            tc.tile_pool(name="big", bufs=1) as big,
            tc.tile_pool(name="xt", bufs=1) as xtp,
            tc.tile_pool(name="pt", bufs=24) as ptp,
            tc.tile_pool(name="recip", bufs=2) as rcp,
            tc.tile_pool(name="rbc", bufs=2) as rbcp,
            tc.tile_pool(name="ysb", bufs=4) as ysbp,
        ):
            # ---- input DMAs: x as 8 half-tiles on 2 queues (first halves
            # of all 4 ct tiles first, so the kq c0 chunks start early) ----
            xt = [xtp.tile([128, N], F16, tag=f"x{ct}", name=f"x{ct}")
                  for ct in range(4)]
            for ct in range(4):
                eng = nc.sync if ct % 2 == 0 else nc.scalar
                eng.dma_start(out=xt[ct][:], in_=xT_d[bass.ts(ct, 128), :])
            wq_t, wk_t, wv_t = [], [], []
            for lst, nm, src in ((wq_t, "wq", wqT_d), (wk_t, "wk", wkT_d),
                                 (wv_t, "wv", wvT_d)):
                for ct in range(4):
                    t = const.tile([128, CS], F16, tag=f"{nm}{ct}",
                                   name=f"{nm}{ct}")
                    nc.gpsimd.dma_start(out=t[:], in_=src[bass.ts(ct, 128), :])
                    lst.append(t)
            bq_sb = const.tile([128, 2], F32, tag="bq", name="bq")
            nc.gpsimd.dma_start(out=bq_sb[:], in_=bq_d[:])
            bk_sb = const.tile([128, 2], F32, tag="bk", name="bk")
            nc.gpsimd.dma_start(out=bk_sb[:], in_=bk_d[:])
            # v bias pre-broadcast host-side so v tiles need no bias matmul
            bvb = const.tile([128, CS], F16, tag="bvb", name="bvb")
            nc.gpsimd.dma_start(out=bvb[:], in_=bvb_d[:])
            wp_t = []
            for j in range(2):
                t = const.tile([128, C], F16, tag=f"wp{j}", name=f"wp{j}")
                nc.gpsimd.dma_start(out=t[:], in_=wpT_d[bass.ts(j, 128), :])
                wp_t.append(t)

            # ---- persistent activations ---------------------------------
            qT_c = [big.tile([128, N], F16, tag=f"qT{j}", name=f"qT{j}")
                    for j in range(2)]
            kT_c = [big.tile([128, N], F16, tag=f"kT{j}", name=f"kT{j}")
                    for j in range(2)]
            v1m = [big.tile([128, HPC, HD + 1], F16, tag=f"v1m_{m}",
                            name=f"v1m_{m}") for m in range(MT)]
            for m in range(MT):
                nc.gpsimd.dma_start(
                    out=v1m[m][:, :, HD:HD + 1],
                    in_=ones_col_d[:, :].rearrange("p (h o) -> p h o", o=1),
                )
            oT_pair = [big.tile([128, N], F16, tag=f"oT{j}", name=f"oT{j}")
                       for j in range(2)]
            y0 = [big.tile([128, N], F16, tag=f"y0_{jj}", name=f"y0_{jj}")
                  for jj in range(4)]

            # kq chunk: one [128,1024] column chunk of q^T or k^T for head
            # pair j, accumulated over the 4 input-channel tiles + bias.
            def kq_chunk(pool, w_t, b_sb, dst, j, c):
                ps = pool.tile([128, 1024], F32, tag="kq", name="kq")
                for ct in range(4):
                    for hf in range(2):
                        nc.tensor.matmul(
                            ps[:, bass.ts(hf, 512)],
                            lhsT=w_t[ct][:, bass.ts(j, 128)],
                            rhs=xt[ct][:, c * 1024 + hf * 512:
                                       c * 1024 + hf * 512 + 512],
                            start=(ct == 0), stop=(ct == 3),
                        )
                nc.vector.tensor_scalar_add(
                    dst[j][:, bass.ts(c, 1024)], ps[:], b_sb[:, j:j + 1])

            # v tile m: [128 tok, 256 ch], bias added in the PSUM->SBUF
            # copy (bvb is the host-pre-broadcast v bias).
            def v_tile(pool, m):
                vp = pool.tile([128, 1024], F32, tag="kq", name="kq")
                for ct in range(4):
                    nc.tensor.matmul(
                        vp[:, 0:CS],
                        lhsT=xt[ct][:, bass.ts(m, 128)],
                        rhs=wv_t[ct][:],
                        start=(ct == 0), stop=(ct == 3),
                    )
                nc.vector.tensor_add(v1m[m][:, :, 0:HD], vp[:, 0:CS],
                                     bvb[:])

            def scores_exp(h, p2, m):
                j, hh = h // 2, h % 2
                psl = slice(hh * 64, hh * 64 + 64)
                sT = stps.tile([128, 1024], F32, tag="sT", name="sT")
                for hf in range(2):
                    nc.tensor.matmul(
                        sT[:, bass.ts(hf, 512)],
                        lhsT=kT_c[j][psl, bass.ts(m, 128)],
                        rhs=qT_c[j][psl, p2 * 1024 + hf * 512:
                                    p2 * 1024 + hf * 512 + 512],
                        start=True, stop=True,
                    )
                pT = ptp.tile([128, 1024], F16, tag="pT", name="pT")
                nc.scalar.activation(
                    out=pT[:], in_=sT[:],
                    func=mybir.ActivationFunctionType.Exp,
                    scale=SCALE,
                )
                return pT

            def av(oT, h, m, pT, first, last):
                for hf in range(2):
                    nc.tensor.matmul(
                        oT[:, bass.ts(hf, 512)],
                        lhsT=v1m[m][:, h, :],
                        rhs=pT[:, bass.ts(hf, 512)],
                        start=first, stop=last,
                    )

            def chain(oT, h, p2):
                j, hh = h // 2, h % 2
                psl = slice(hh * 64, hh * 64 + 64)
                den = rcp.tile([1, 1024], F32, tag="den", name="den")
                nc.vector.tensor_copy(den[:], oT[HD:HD + 1, :])
                rc = rcp.tile([1, 1024], F32, tag="rc", name="rc")
                nc.vector.reciprocal_approx_fast(rc[:], den[:])
                bc = rbcp.tile([64, 1024], F32, tag="bc", name="bc")
                nc.gpsimd.partition_broadcast(bc[:], rc[:])
                nc.vector.tensor_mul(
                    oT_pair[j][psl, bass.ts(p2, 1024)], oT[0:HD, :], bc[:])

            units = [(h, p2) for h in range(HPC) for p2 in range(2)]
            pT_ring = {}

            kqps = tc.alloc_tile_pool(name="kqps", bufs=2, space="PSUM",
                                      side="right")
            kq_chunk(kqps, wq_t, bq_sb, qT_c, 0, 0)
            kq_chunk(kqps, wk_t, bk_sb, kT_c, 0, 0)
            kq_chunk(kqps, wq_t, bq_sb, qT_c, 0, 1)
            kq_chunk(kqps, wk_t, bk_sb, kT_c, 0, 1)

            stps = tc.alloc_tile_pool(name="stps", bufs=2, space="PSUM",
                                      side="left")
            # ---- unit 0 (h0,p2=0): fillers = v tiles --------------------
            vq = list(range(MT))  # v tiles still to emit
            pT_ring[0] = []
            for m in range(MT):
                pT_ring[0].append(scores_exp(0, 0, m))
                if vq:
                    v_tile(kqps, vq.pop(0))
            # ---- unit 1 (h0,p2=1): fillers = kq_j1 bursts + v rest ------
            kq1 = [(wq_t, bq_sb, qT_c, 1, 0), (wk_t, bk_sb, kT_c, 1, 0),
                   (wq_t, bq_sb, qT_c, 1, 1), (wk_t, bk_sb, kT_c, 1, 1)]
            pT_ring[1] = []
            for m in range(MT):
                pT_ring[1].append(scores_exp(0, 1, m))
                if m % 4 == 1:
                    w_t, b_sb, dst, j, c = kq1[m // 4]
                    kq_chunk(kqps, w_t, b_sb, dst, j, c)
                elif vq:
                    v_tile(kqps, vq.pop(0))
            kqps.release()

            otps = tc.alloc_tile_pool(name="otps", bufs=2, space="PSUM",
                                      side="right")
            # ---- units 2..7: scores/exp + AV of unit u-2 (lag) ----------
            for u in range(2, 8):
                h, p2 = units[u]
                ph, pp2 = units[u - 2]
                oT_prev = otps.tile([HD + 1, 1024], F32, tag="oT", name="oT")
                pT_ring[u] = []
                for m in range(MT):
                    pT_ring[u].append(scores_exp(h, p2, m))
                    av(oT_prev, ph, m, pT_ring[u - 2][m],
                       first=(m == 0), last=(m == MT - 1))
                chain(oT_prev, ph, pp2)
            stps.release()

            # ---- tail: AVs for units 6,7, chains, 2-pass projection -----
            for u in (6, 7):
                h, p2 = units[u]
                oT = otps.tile([HD + 1, 1024], F32, tag="oT", name="oT")
                for m in range(MT):
                    av(oT, h, m, pT_ring[u][m],
                       first=(m == 0), last=(m == MT - 1))
                chain(oT, h, p2)
            otps.release()

            yps = tc.alloc_tile_pool(name="yps", bufs=4, space="PSUM",
                                     side="left")
            # pass 1 (j=0): MM + gpsimd copy (keeps DVE free for pass-2 adds)
            for jj in range(4):
                for c2 in range(4):
                    yp = yps.tile([128, 512], F32, tag="yp", name="yp")
                    nc.tensor.matmul(
                        yp[:],
                        lhsT=wp_t[0][:, bass.ts(jj, 128)],
                        rhs=oT_pair[0][:, bass.ts(c2, 512)],
                        start=True, stop=True,
                    )
                    nc.vector.tensor_copy(
                        y0[jj][:, bass.ts(c2, 512)], yp[:])
            # pass 2 (j=1): MM + DVE add + DMA out on 2 queues
            for jj in range(4):
                for c2 in range(4):
                    yp = yps.tile([128, 512], F32, tag="yp", name="yp")
                    nc.tensor.matmul(
                        yp[:],
                        lhsT=wp_t[1][:, bass.ts(jj, 128)],
                        rhs=oT_pair[1][:, bass.ts(c2, 512)],
                        start=True, stop=True,
                    )
                    ys = ysbp.tile([128, 512], F16, tag="ys", name="ys")
                    nc.vector.tensor_add(
                        ys[:], y0[jj][:, bass.ts(c2, 512)], yp[:])
                    eng = nc.sync if c2 % 2 == 0 else nc.scalar
                    eng.dma_start(
                        out=yT_d[bass.ts(jj, 128), bass.ts(c2, 512)],
                        in_=ys[:])
            yps.release()

    nc.compile()
    return nc


def get_nc():
    global _NC
    if _NC is None:
        _NC = _build()
    return _NC


def shard_inputs(x, w_qkv, b_qkv, w_proj, b_proj):
    x = np.asarray(x, dtype=np.float32)
    w_qkv = np.asarray(w_qkv, dtype=np.float32)
    b_qkv = np.asarray(b_qkv, dtype=np.float32)
    w_proj = np.asarray(w_proj, dtype=np.float32)
    ones_col = np.ones((128, HPC), np.float16)
    in_maps = []
    for core in range(NCORES):
        b, g = core // 2, core % 2
        sl = slice(g * CS, (g + 1) * CS)
        in_maps.append({
            "xT": np.ascontiguousarray(x[b].T).astype(np.float16),
            "wqT": np.ascontiguousarray(w_qkv[sl, :].T).astype(np.float16),
            "wkT": np.ascontiguousarray(w_qkv[C:][sl, :].T).astype(np.float16),
            "wvT": np.ascontiguousarray(
                w_qkv[2 * C:][sl, :].T).astype(np.float16),
            "wpT": np.ascontiguousarray(w_proj[:, sl].T).astype(np.float16),
            "bq": np.ascontiguousarray(b_qkv[sl].reshape(2, 128).T),
            "bk": np.ascontiguousarray(b_qkv[C:][sl].reshape(2, 128).T),
            "bvb": np.tile(b_qkv[2 * C:][sl].reshape(1, CS),
                           (128, 1)).astype(np.float16),
            "ones_col": ones_col,
        })
    return in_maps


def gather_output(results, b_proj):
    b_proj = np.asarray(b_proj, dtype=np.float32)
    out = np.empty((B, N, C), np.float32)
    for b in range(B):
        yT = (results[2 * b]["yT"].astype(np.float32)
              + results[2 * b + 1]["yT"].astype(np.float32))
        out[b] = yT.T + b_proj[None, :]
    return out


def kernel(x, w_qkv, b_qkv, w_proj, b_proj):
    nc = get_nc()
    in_maps = shard_inputs(x, w_qkv, b_qkv, w_proj, b_proj)
    res = run_bass_kernel_spmd(nc, in_maps, core_ids=list(range(NCORES)))
    return gather_output(res.results, b_proj)


# revision 29
# speedup vs baseline: 1.7749x; 1.0200x over previous
"""Multi-head self-attention (B=4, N=2048, C=512, H=8) on 8 trn2 NeuronCores.

Sharding: core = 2*b + g  (b = batch, g = head-half).  Each core handles one
batch element and 4 heads (channel slice of 256), computes its partial output
projection y^T = W_p[:, slice] @ out[slice], and the host sums the two
partials per batch element (fp32) and adds b_proj.

All matmul operands are fp16 (PSUM accumulation fp32).  The exp stream on
the ACT engine (128 x [128,1024] tiles, ~1.1us each) is the throughput
floor; emission keeps ACT saturated from ~15us and the PE continuously fed:

  q/k head-pair 0, column chunk 0 only               [kqps pool, PSUM right]
  unit (h0,p2=0): scores+exp, fillers: kq_j0 chunk 1 + v tiles
  unit (h0,p2=1): scores+exp, fillers: kq_j1 chunks  [all in kqps tiles]
  kqps releases -> otps opens (right)
  units 2..7: scores+exp + AV of unit u-2 (lag-2, deep pT ring)
  tail: AV(u6)+chain, AV(u7)+chain, then 2-pass projection: pass1 j=0
  (ACT Copy to SBUF - same ACT table as Exp), pass2 j=1 (DVE add), fp16
  output on 2 DMA queues.
"""

import numpy as np

import concourse.bacc as bacc
import concourse.bass as bass
import concourse.mybir as mybir
import concourse.tile as tile
from concourse.bass_utils import run_bass_kernel_spmd

B, N, C, H, HD = 4, 2048, 512, 8, 64
HPC, CS = 4, 256  # heads per core, channels per core
SCALE = HD ** -0.5
F16 = mybir.dt.float16
F32 = mybir.dt.float32
NCORES = 8
MT = N // 128  # 16 key tiles

_NC = None


def _build():
    nc = bacc.Bacc("TRN2", target_bir_lowering=False, debug=False,
                   num_devices=NCORES)
    xT_d = nc.dram_tensor("xT", [C, N], F16, kind="ExternalInput")
    wqT_d = nc.dram_tensor("wqT", [C, CS], F16, kind="ExternalInput")
    wkT_d = nc.dram_tensor("wkT", [C, CS], F16, kind="ExternalInput")
    wvT_d = nc.dram_tensor("wvT", [C, CS], F16, kind="ExternalInput")
    wpT_d = nc.dram_tensor("wpT", [CS, C], F16, kind="ExternalInput")
    bq_d = nc.dram_tensor("bq", [128, 2], F32, kind="ExternalInput")
    bk_d = nc.dram_tensor("bk", [128, 2], F32, kind="ExternalInput")
    bvb_d = nc.dram_tensor("bvb", [128, CS], F16, kind="ExternalInput")
    ones_col_d = nc.dram_tensor("ones_col", [128, HPC], F16,
                                kind="ExternalInput")
    yT_d = nc.dram_tensor("yT", [C, N], F16, kind="ExternalOutput")

    with tile.TileContext(nc) as tc:
        with (
            tc.tile_pool(name="const", bufs=1) as const,
            tc.tile_pool(name="big", bufs=1) as big,
            tc.tile_pool(name="xt", bufs=1) as xtp,
            tc.tile_pool(name="pt", bufs=24) as ptp,
            tc.tile_pool(name="recip", bufs=2) as rcp,
            tc.tile_pool(name="rbc", bufs=2) as rbcp,
            tc.tile_pool(name="ysb", bufs=4) as ysbp,
        ):
            # ---- input DMAs: x as 8 half-tiles on 2 queues (first halves
            # of all 4 ct tiles first, so the kq c0 chunks start early) ----
            xt = [xtp.tile([128, N], F16, tag=f"x{ct}", name=f"x{ct}")
                  for ct in range(4)]
            for ct in range(4):
                eng = nc.sync if ct % 2 == 0 else nc.scalar
                eng.dma_start(out=xt[ct][:], in_=xT_d[bass.ts(ct, 128), :])
            wq_t, wk_t, wv_t = [], [], []
            for lst, nm, src in ((wq_t, "wq", wqT_d), (wk_t, "wk", wkT_d),
                                 (wv_t, "wv", wvT_d)):
                for ct in range(4):
                    t = const.tile([128, CS], F16, tag=f"{nm}{ct}",
                                   name=f"{nm}{ct}")
                    nc.gpsimd.dma_start(out=t[:], in_=src[bass.ts(ct, 128), :])
                    lst.append(t)
            bq_sb = const.tile([128, 2], F32, tag="bq", name="bq")
            nc.gpsimd.dma_start(out=bq_sb[:], in_=bq_d[:])
            bk_sb = const.tile([128, 2], F32, tag="bk", name="bk")
            nc.gpsimd.dma_start(out=bk_sb[:], in_=bk_d[:])
            # v bias pre-broadcast host-side so v tiles need no bias matmul
            bvb = const.tile([128, CS], F16, tag="bvb", name="bvb")
            nc.gpsimd.dma_start(out=bvb[:], in_=bvb_d[:])
            wp_t = []
            for j in range(2):
                t = const.tile([128, C], F16, tag=f"wp{j}", name=f"wp{j}")
                nc.gpsimd.dma_start(out=t[:], in_=wpT_d[bass.ts(j, 128), :])
                wp_t.append(t)

            # ---- persistent activations ---------------------------------
            # q^T/k^T as separate tiles per column chunk c so a chunk can
            # be produced while earlier chunks are already being read
            # (same-tile write-after-read races on hardware).
            qT_c = [[big.tile([128, 1024], F16, tag=f"qT{j}_{c}",
                              name=f"qT{j}_{c}") for c in range(2)]
                    for j in range(2)]
            kT_c = [[big.tile([128, 1024], F16, tag=f"kT{j}_{c}",
                              name=f"kT{j}_{c}") for c in range(2)]
                    for j in range(2)]
            v1m = [big.tile([128, HPC, HD + 1], F16, tag=f"v1m_{m}",
                            name=f"v1m_{m}") for m in range(MT)]
            for m in range(MT):
                nc.gpsimd.dma_start(
                    out=v1m[m][:, :, HD:HD + 1],
                    in_=ones_col_d[:, :].rearrange("p (h o) -> p h o", o=1),
                )
            oT_pair = [big.tile([128, N], F16, tag=f"oT{j}", name=f"oT{j}")
                       for j in range(2)]
            y0 = [big.tile([128, N], F16, tag=f"y0_{jj}", name=f"y0_{jj}")
                  for jj in range(4)]

            # kq chunk: one [128,1024] column chunk of q^T or k^T for head
            # pair j, accumulated over the 4 input-channel tiles + bias.
            def kq_chunk(pool, w_t, b_sb, dst, j, c):
                ps = pool.tile([128, 1024], F32, tag="kq", name="kq")
                for ct in range(4):
                    for hf in range(2):
                        nc.tensor.matmul(
                            ps[:, bass.ts(hf, 512)],
                            lhsT=w_t[ct][:, bass.ts(j, 128)],
                            rhs=xt[ct][:, c * 1024 + hf * 512:
                                       c * 1024 + hf * 512 + 512],
                            start=(ct == 0), stop=(ct == 3),
                        )
                nc.vector.tensor_scalar_add(
                    dst[j][c][:], ps[:], b_sb[:, j:j + 1])

            # v tile m: [128 tok, 256 ch], bias added in the PSUM->SBUF
            # copy (bvb is the host-pre-broadcast v bias).
            def v_tile(pool, m):
                vp = pool.tile([128, 1024], F32, tag="kq", name="kq")
                for ct in range(4):
                    nc.tensor.matmul(
                        vp[:, 0:CS],
                        lhsT=xt[ct][:, bass.ts(m, 128)],
                        rhs=wv_t[ct][:],
                        start=(ct == 0), stop=(ct == 3),
                    )
                nc.vector.tensor_add(v1m[m][:, :, 0:HD], vp[:, 0:CS],
                                     bvb[:])

            def scores_exp(h, p2, m):
                j, hh = h // 2, h % 2
                psl = slice(hh * 64, hh * 64 + 64)
                sT = stps.tile([128, 1024], F32, tag="sT", name="sT")
                for hf in range(2):
                    nc.tensor.matmul(
                        sT[:, bass.ts(hf, 512)],
                        lhsT=kT_c[j][m // 8][psl, bass.ts(m % 8, 128)],
                        rhs=qT_c[j][p2][psl, bass.ts(hf, 512)],
                        start=True, stop=True,
                    )
                pT = ptp.tile([128, 1024], F16, tag="pT", name="pT")
                nc.scalar.activation(
                    out=pT[:], in_=sT[:],
                    func=mybir.ActivationFunctionType.Exp,
                    scale=SCALE,
                )
                return pT

            def av(oT, h, m, pT, first, last):
                for hf in range(2):
                    nc.tensor.matmul(
                        oT[:, bass.ts(hf, 512)],
                        lhsT=v1m[m][:, h, :],
                        rhs=pT[:, bass.ts(hf, 512)],
                        start=first, stop=last,
                    )

            def chain(oT, h, p2):
                j, hh = h // 2, h % 2
                psl = slice(hh * 64, hh * 64 + 64)
                den = rcp.tile([1, 1024], F32, tag="den", name="den")
                nc.vector.tensor_copy(den[:], oT[HD:HD + 1, :])
                rc = rcp.tile([1, 1024], F32, tag="rc", name="rc")
                nc.vector.reciprocal_approx_fast(rc[:], den[:])
                bc = rbcp.tile([64, 1024], F32, tag="bc", name="bc")
                nc.gpsimd.partition_broadcast(bc[:], rc[:])
                nc.vector.tensor_mul(
                    oT_pair[j][psl, bass.ts(p2, 1024)], oT[0:HD, :], bc[:])

            units = [(h, p2) for h in range(HPC) for p2 in range(2)]
            pT_ring = {}

            kqps = tc.alloc_tile_pool(name="kqps", bufs=2, space="PSUM",
                                      side="right")
            kq_chunk(kqps, wq_t, bq_sb, qT_c, 0, 0)
            kq_chunk(kqps, wk_t, bk_sb, kT_c, 0, 0)

            stps = tc.alloc_tile_pool(name="stps", bufs=2, space="PSUM",
                                      side="left")
            # ---- unit 0 (h0,p2=0): fillers = kq_j0 chunk-1 bursts + v ---
            # (chunk 1 lands in its own tiles, first read at m=8 / unit 1)
            vq = list(range(MT))  # v tiles still to emit
            pT_ring[0] = []
            for m in range(MT):
                pT_ring[0].append(scores_exp(0, 0, m))
                if m == 1:
                    kq_chunk(kqps, wq_t, bq_sb, qT_c, 0, 1)
                elif m == 4:
                    kq_chunk(kqps, wk_t, bk_sb, kT_c, 0, 1)
                elif vq:
                    v_tile(kqps, vq.pop(0))
            # ---- unit 1 (h0,p2=1): fillers = kq_j1 bursts + v rest ------
            kq1 = [(wq_t, bq_sb, qT_c, 1, 0), (wk_t, bk_sb, kT_c, 1, 0),
                   (wq_t, bq_sb, qT_c, 1, 1), (wk_t, bk_sb, kT_c, 1, 1)]
            pT_ring[1] = []
            for m in range(MT):
                pT_ring[1].append(scores_exp(0, 1, m))
                if m % 4 == 1:
                    w_t, b_sb, dst, j, c = kq1[m // 4]
                    kq_chunk(kqps, w_t, b_sb, dst, j, c)
                elif vq:
                    v_tile(kqps, vq.pop(0))
            kqps.release()

            otps = tc.alloc_tile_pool(name="otps", bufs=2, space="PSUM",
                                      side="right")
            # ---- units 2..7: scores/exp + AV of unit u-2 (lag) ----------
            for u in range(2, 8):
                h, p2 = units[u]
                ph, pp2 = units[u - 2]
                oT_prev = otps.tile([HD + 1, 1024], F32, tag="oT", name="oT")
                pT_ring[u] = []
                for m in range(MT):
                    pT_ring[u].append(scores_exp(h, p2, m))
                    av(oT_prev, ph, m, pT_ring[u - 2][m],
                       first=(m == 0), last=(m == MT - 1))
                chain(oT_prev, ph, pp2)
            stps.release()

            # ---- tail: AVs for units 6,7, chains, 2-pass projection -----
            for u in (6, 7):
                h, p2 = units[u]
                oT = otps.tile([HD + 1, 1024], F32, tag="oT", name="oT")
                for m in range(MT):
                    av(oT, h, m, pT_ring[u][m],
                       first=(m == 0), last=(m == MT - 1))
                chain(oT, h, p2)
            otps.release()

            yps = tc.alloc_tile_pool(name="yps", bufs=4, space="PSUM",
                                     side="left")
            # pass 1 (j=0): MM + gpsimd copy (keeps DVE free for pass-2 adds)
            for jj in range(4):
                for c2 in range(4):
                    yp = yps.tile([128, 512], F32, tag="yp", name="yp")
                    nc.tensor.matmul(
                        yp[:],
                        lhsT=wp_t[0][:, bass.ts(jj, 128)],
                        rhs=oT_pair[0][:, bass.ts(c2, 512)],
                        start=True, stop=True,
                    )
                    nc.scalar.activation(
                        out=y0[jj][:, bass.ts(c2, 512)], in_=yp[:],
                        func=mybir.ActivationFunctionType.Copy,
                    )
            # pass 2 (j=1): MM + DVE add + DMA out on 2 queues
            for jj in range(4):
                for c2 in range(4):
                    yp = yps.tile([128, 512], F32, tag="yp", name="yp")
                    nc.tensor.matmul(
                        yp[:],
                        lhsT=wp_t[1][:, bass.ts(jj, 128)],
                        rhs=oT_pair[1][:, bass.ts(c2, 512)],
                        start=True, stop=True,
                    )
                    ys = ysbp.tile([128, 512], F16, tag="ys", name="ys")
                    nc.vector.tensor_add(
                        ys[:], y0[jj][:, bass.ts(c2, 512)], yp[:])
                    eng = nc.sync if c2 % 2 == 0 else nc.scalar
                    eng.dma_start(
                        out=yT_d[bass.ts(jj, 128), bass.ts(c2, 512)],
                        in_=ys[:])
            yps.release()

    nc.compile()
    return nc


def get_nc():
    global _NC
    if _NC is None:
        _NC = _build()
    return _NC


def shard_inputs(x, w_qkv, b_qkv, w_proj, b_proj):
    x = np.asarray(x, dtype=np.float32)
    w_qkv = np.asarray(w_qkv, dtype=np.float32)
    b_qkv = np.asarray(b_qkv, dtype=np.float32)
    w_proj = np.asarray(w_proj, dtype=np.float32)
    ones_col = np.ones((128, HPC), np.float16)
    in_maps = []
    for core in range(NCORES):
        b, g = core // 2, core % 2
        sl = slice(g * CS, (g + 1) * CS)
        in_maps.append({
            "xT": np.ascontiguousarray(x[b].T).astype(np.float16),
            "wqT": np.ascontiguousarray(w_qkv[sl, :].T).astype(np.float16),
            "wkT": np.ascontiguousarray(w_qkv[C:][sl, :].T).astype(np.float16),
            "wvT": np.ascontiguousarray(
                w_qkv[2 * C:][sl, :].T).astype(np.float16),
            "wpT": np.ascontiguousarray(w_proj[:, sl].T).astype(np.float16),
            "bq": np.ascontiguousarray(b_qkv[sl].reshape(2, 128).T),
            "bk": np.ascontiguousarray(b_qkv[C:][sl].reshape(2, 128).T),
            "bvb": np.tile(b_qkv[2 * C:][sl].reshape(1, CS),
                           (128, 1)).astype(np.float16),
            "ones_col": ones_col,
        })
    return in_maps


def gather_output(results, b_proj):
    b_proj = np.asarray(b_proj, dtype=np.float32)
    out = np.empty((B, N, C), np.float32)
    for b in range(B):
        yT = (results[2 * b]["yT"].astype(np.float32)
              + results[2 * b + 1]["yT"].astype(np.float32))
        out[b] = yT.T + b_proj[None, :]
    return out


def kernel(x, w_qkv, b_qkv, w_proj, b_proj):
    nc = get_nc()
    in_maps = shard_inputs(x, w_qkv, b_qkv, w_proj, b_proj)
    res = run_bass_kernel_spmd(nc, in_maps, core_ids=list(range(NCORES)))
    return gather_output(res.results, b_proj)
